# revision 34
# baseline (speedup 1.0000x reference)
"""CaptionNet (attention + 2-LSTM) Trainium2 kernel, 8 NeuronCores.

Exploits:
- attention softmax over a size-1 axis == 1.0 exactly -> context == image_vectors
- LSTM2 uses h1 as input AND state -> W2 = W_ih2 + W_hh2 folds into one matmul
- lstm2's cell state input is c1 and the carry c is c2 -> single cell buffer
- word-emb / image parts of the LSTM1 input products are precomputed batched

Sharding: H sharded 8-way in the recurrent loop (h chunks exchanged via
AllGather twice/step); vocab projection sharded 8-way over V; weights
pre-transposed/pre-cast to bf16 on the host (pure layout/sharding work).

Scheduling: input DMAs are grouped into three completion sets so the PE
starts the preamble before the loop weights finish loading; the vocab
projection is split around the two AllGather windows; gate activations
overlap (tanh(g) runs while f*c computes); output evacuation splits
across Scalar and Vector engines.
"""

import contextlib
import numpy as np
import ml_dtypes
import concourse.bass as bass
import concourse.mybir as mybir
from concourse.bass_utils import run_bass_kernel_spmd

B, T, V, E, H, F = 128, 24, 12000, 512, 1024, 2048
NC = 8
HC = H // NC          # 128
G = 4 * HC            # 512 gate rows per core (i,f,o,g reordered)
VC = V // NC          # 1500
KT = H // 128         # 8
ET = E // 128         # 4
FT = F // 128         # 16
NT = B * T // 128     # 24
F32 = mybir.dt.float32
BF16 = mybir.dt.bfloat16
AF = mybir.ActivationFunctionType
ALU = mybir.AluOpType
BF = ml_dtypes.bfloat16


def _kchunks(wT, n_free):
    """[K, n] -> [128, (K//128)*n]; [p, k*n+j] = wT[k*128+p, j]."""
    K = wT.shape[0]
    return np.ascontiguousarray(
        wT.reshape(K // 128, 128, n_free).transpose(1, 0, 2).reshape(128, -1))


def _build(nc, T_loop=T):
    def inp(name, shape, dt):
        return nc.dram_tensor(name, list(shape), dt, kind="ExternalInput").ap()

    whh1T = inp("whh1T", [128, KT * G], BF16).rearrange("p (k g) -> p k g", k=KT)
    w2T = inp("w2T", [128, KT * G], BF16).rearrange("p (k g) -> p k g", k=KT)
    woutT = inp("woutT", [128, KT * VC], BF16).rearrange("p (k v) -> p k v", k=KT)
    weT = inp("weT", [128, ET * G], BF16).rearrange("p (k g) -> p k g", k=ET)
    wfT = inp("wfT", [128, FT * G], BF16).rearrange("p (k g) -> p k g", k=FT)
    wimgT = inp("wimgT", [128, FT * H], BF16).rearrange("p (k h) -> p k h", k=FT)
    wimgcT = inp("wimgcT", [128, FT * 128], BF16).rearrange(
        "p (k h) -> p k h", k=FT)
    ivT = inp("ivT", [128, FT * 128], BF16).rearrange("p (k b) -> p k b", k=FT)
    capT = inp("capT", [128, ET * B * T], BF16).rearrange(
        "p (k n) -> p k n", k=ET)
    b1b = inp("b1b", [128, G], BF16)
    b2b = inp("b2b", [128, G], BF16)
    bimgb = inp("bimgb", [128, H], BF16)
    bimgcb = inp("bimgcb", [128, 128], BF16)
    boutb = inp("boutb", [128, VC], BF16)
    idn = inp("idn", [128, 128], BF16)
    idnf = inp("idnf", [128, 128], F32)
    y = nc.dram_tensor("y", [B * T, VC], F32, kind="ExternalOutput").ap()
    bin1 = nc.dram_tensor("bin1", [128, B], BF16, kind="Internal").ap()
    bout1 = nc.dram_tensor("bout1", [H, B], BF16, kind="Internal",
                           addr_space="Shared").ap()
    bin2 = nc.dram_tensor("bin2", [128, B], BF16, kind="Internal").ap()
    bout2 = nc.dram_tensor("bout2", [H, B], BF16, kind="Internal",
                           addr_space="Shared").ap()
    dbin = nc.dram_tensor("dbin", [128, 8], BF16, kind="Internal").ap()
    dbout = nc.dram_tensor("dbout", [128 * NC, 8], BF16, kind="Internal",
                           addr_space="Shared").ap()

    PE, ACT, DVE, SP, PL = nc.tensor, nc.scalar, nc.vector, nc.sync, nc.gpsimd
    ctx = contextlib.ExitStack()
    sb = lambda n, s, d: ctx.enter_context(nc.sbuf_tensor(n, s, d))
    ps = lambda n, s, d: ctx.enter_context(nc.psum_tensor(n, s, d))
    sem = lambda n: ctx.enter_context(nc.semaphore(n))

    # persistent SBUF
    s_whh1 = sb("s_whh1", [128, KT, G], BF16)
    s_w2 = sb("s_w2", [128, KT, G], BF16)
    s_wout = sb("s_wout", [128, KT, VC], BF16)
    s_pre = sb("s_pre", [128, NT, G], BF16)
    s_b2b = sb("s_b2b", [128, G], BF16)
    s_boutb = sb("s_boutb", [128, VC], BF16)
    s_idn = sb("s_idn", [128, 128], BF16)
    s_idnf = sb("s_idnf", [128, 128], F32)
    s_h1T = sb("s_h1T", [128, KT, 128], BF16)
    s_h2T = [sb(f"s_h2T{i}", [128, KT, 128], BF16) for i in range(2)]
    s_c = sb("s_c", [128, HC], F32)
    s_sig = sb("s_sig", [128, 384], F32)
    s_tg = sb("s_tg", [128, HC], F32)
    s_th = sb("s_th", [128, HC], F32)
    s_tA = sb("s_tA", [128, HC], F32)
    s_tB = sb("s_tB", [128, HC], F32)
    s_h = sb("s_h", [128, HC], F32)
    s_hcT1 = sb("s_hcT1", [128, 128], BF16)
    s_hcT2 = sb("s_hcT2", [128, 128], BF16)
    s_out = sb("s_out", [128, VC], F32)
    # kept live through the loop for in-loop pre-tile computation
    s_cap = sb("s_cap", [128, ET, B * T], BF16)
    s_we = sb("s_we", [128, ET, G], BF16)
    s_img = sb("s_img", [128, G], BF16)

    s_ldA1 = sem("s_ldA1"); s_ldA2 = sem("s_ldA2"); s_ldB = sem("s_ldB")
    s_ldC = sem("s_ldC"); s_ldD = sem("s_ldD")
    s_bh0 = sem("s_bh0"); s_bh0ev = sem("s_bh0ev")
    s_bimg = sem("s_bimg"); s_bimgev = sem("s_bimgev")
    s_bc0 = sem("s_bc0"); s_bc0ev = sem("s_bc0ev")
    s_bh0T = sem("s_bh0T"); s_bh0Tev = sem("s_bh0Tev")
    s_preMM = sem("s_preMM")
    s_preEvA = sem("s_preEvA"); s_preEvD = sem("s_preEvD")
    s_g1 = sem("s_g1"); s_sA1 = sem("s_sA1"); s_sB1 = sem("s_sB1")
    s_c1 = sem("s_c1"); s_th1 = sem("s_th1"); s_h1 = sem("s_h1")
    s_tp1 = sem("s_tp1"); s_ev1 = sem("s_ev1"); s_din1 = sem("s_din1")
    s_ag1 = sem("s_ag1"); s_hT1 = sem("s_hT1")
    s_g2 = sem("s_g2"); s_sA2 = sem("s_sA2"); s_sB2 = sem("s_sB2")
    s_c2 = sem("s_c2"); s_th2 = sem("s_th2"); s_h2 = sem("s_h2")
    s_tp2 = sem("s_tp2"); s_ev2 = sem("s_ev2"); s_din2 = sem("s_din2")
    s_ag2 = sem("s_ag2"); s_hT2 = sem("s_hT2")
    s_op = sem("s_op"); s_odma = sem("s_odma")
    s_oevA = sem("s_oevA"); s_oevD = sem("s_oevD")

    s_ag0 = sem("s_ag0")
    # fire dummy AllGathers immediately: the ncfw init barrier (~50us) and
    # the slow post-barrier warmup overlap the input loads instead of
    # blocking the loop's first real AG
    for _ in range(3):
        PL.collective_compute(
            "AllGather", ALU.bypass, replica_groups=[list(range(NC))],
            ins=[dbin.opt()], outs=[dbout.opt()]).then_inc(s_ag0, 1)

    counts = {}
    def load(dst, src, s):
        SP.dma_start(dst, src).then_inc(s, 16)
        counts[s.name] = counts.get(s.name, 0) + 16

    # group A1: minimal set for the h0 matmuls
    load(s_idn[:], idn, s_ldA1)
    load(s_idnf[:], idnf, s_ldA1)

    # ---------------- preamble ----------------
    with (
        nc.sbuf_tensor("s_wimg", [128, FT, H], BF16) as s_wimg,
        nc.sbuf_tensor("s_wimgc", [128, FT, 128], BF16) as s_wimgc,
        nc.sbuf_tensor("s_ivT", [128, FT, 128], BF16) as s_ivT,
        nc.sbuf_tensor("s_wf", [128, FT, G], BF16) as s_wf,
        nc.sbuf_tensor("s_b1b", [128, G], BF16) as s_b1b,
        nc.sbuf_tensor("s_bimgb", [128, H], BF16) as s_bimgb,
        nc.sbuf_tensor("s_bimgcb", [128, 128], BF16) as s_bimgcb,
        nc.sbuf_tensor("s_h0", [128, H], F32) as s_h0,
        nc.psum_tensor("p_h0", [128, H], F32) as p_h0,
        nc.psum_tensor("p_pre", [128, 4, G], F32) as p_pre,
    ):
        load(s_ivT[:], ivT, s_ldA1)
        load(s_wimg[:], wimgT, s_ldA1)
        load(s_bimgb[:], bimgb, s_ldA1)
        # group A2: c0 / img-part inputs
        load(s_wimgc[:], wimgcT, s_ldA2)
        load(s_bimgcb[:], bimgcb, s_ldA2)
        load(s_wf[:], wfT, s_ldA2)
        load(s_b1b[:], b1b, s_ldA2)
        # group D: pre-tile inputs (needed mid-preamble)
        load(s_we[:], weT, s_ldD)
        load(s_cap[:], capT, s_ldD)
        # group B: recurrent loop weights (needed at t=0)
        load(s_whh1[:], whh1T, s_ldB)
        load(s_w2[:], w2T, s_ldB)
        load(s_b2b[:], b2b, s_ldB)
        # group C: vocab projection (first used at t=1)
        load(s_wout[:], woutT, s_ldC)
        load(s_boutb[:], boutb, s_ldC)

        PE.wait_ge(s_ldA1, counts["s_ldA1"])

        # h0 = IV @ W_img.T + b_img (replicated full)
        for nn2 in range(2):
            sl = slice(nn2 * 512, (nn2 + 1) * 512)
            for k in range(FT):
                PE.matmul(p_h0[:, sl], s_ivT[:, k, :], s_wimg[:, k, sl],
                          start=(k == 0), stop=False)
            PE.matmul(p_h0[:, sl], s_idn[:], s_bimgb[:, sl],
                      start=False, stop=True)
        PE.drain().then_inc(s_bh0, 1)
        DVE.wait_ge(s_bh0, 1)
        DVE.tensor_copy(s_h0[:], p_h0[:, :]).then_inc(s_bh0ev, 1)

        PE.wait_ge(s_ldA2, counts["s_ldA2"])

        # c0 chunk = IV @ W_img[chunk].T + b_img[chunk]  (per-core input data)
        for k in range(FT):
            PE.matmul(p_pre[:, 3, 0:128], s_ivT[:, k, :], s_wimgc[:, k, :],
                      start=(k == 0), stop=False)
        PE.matmul(p_pre[:, 3, 0:128], s_idn[:], s_bimgcb[:],
                  start=False, stop=True)
        PE.drain().then_inc(s_bc0, 1)
        DVE.wait_ge(s_bc0, 1)
        DVE.tensor_copy(s_c[:], p_pre[:, 3, 0:128]).then_inc(s_bc0ev, 1)

        # img_part = IV @ WF_c.T + b1  (bank 0)
        for k in range(FT):
            PE.matmul(p_pre[:, 0, :], s_ivT[:, k, :], s_wf[:, k, :],
                      start=(k == 0), stop=False)
        PE.matmul(p_pre[:, 0, :], s_idn[:], s_b1b[:], start=False, stop=True)
        PE.drain().then_inc(s_bimg, 1)
        ACT.wait_ge(s_bimg, 1)
        ACT.activation(s_img[:], p_pre[:, 0, :], AF.Copy).then_inc(s_bimgev, 1)

        # h0T chunks -> s_h2T[1]  (bank 1, serialized via evac sem)
        PE.wait_ge(s_bh0ev, 1)
        for k in range(KT):
            if k > 0:
                PE.wait_ge(s_bh0Tev, k)
            PE.transpose(p_pre[:, 1, 0:128], s_h0[:, k * 128:(k + 1) * 128],
                         s_idnf[:])
            PE.drain().then_inc(s_bh0T, 1)
            DVE.wait_ge(s_bh0T, k + 1)
            DVE.tensor_copy(s_h2T[1][:, k, :], p_pre[:, 1, 0:128]).then_inc(
                s_bh0Tev, 1)

        # pre tiles 0,1 (emb+img+bias); tiles 2..NT-1 move into the loop
        PE.wait_ge(s_bimgev, 1)
        PE.wait_ge(s_ldD, counts["s_ldD"])
        for m in range(2):
            bank = 0 if (m % 2 == 0) else 2
            for k in range(ET):
                PE.matmul(p_pre[:, bank, :],
                          s_cap[:, k, m * 128:(m + 1) * 128],
                          s_we[:, k, :], start=(k == 0), stop=False)
            PE.matmul(p_pre[:, bank, :], s_idn[:], s_img[:],
                      start=False, stop=True)
            PE.drain().then_inc(s_preMM, 1)
            if m % 2 == 0:
                ACT.wait_ge(s_preMM, m + 1)
                ACT.activation(s_pre[:, m, :], p_pre[:, bank, :],
                               AF.Copy).then_inc(s_preEvA, 1)
            else:
                DVE.wait_ge(s_preMM, m + 1)
                DVE.tensor_copy(s_pre[:, m, :], p_pre[:, bank, :]).then_inc(
                    s_preEvD, 1)

    # persistent PSUM
    p_g1 = ps("p_g1", [128, G], F32)
    p_g2 = ps("p_g2", [128, G], F32)
    p_t1 = ps("p_t1", [128, 128], F32)
    p_t2 = ps("p_t2", [128, 128], F32)
    p_op = ps("p_op", [128, 1536], F32)
    p_pl = ps("p_pl", [128, G], F32)
    s_plMM = sem("s_plMM"); s_plEv = sem("s_plEv")
    OPN = [(0, 512), (512, 512), (1024, VC - 1024)]

    def eltwise(psrc, s_gX, s_sAX, s_sBX, s_cX, s_thX, s_hX, t):
        tt = t + 1
        ACT.wait_ge(s_gX, tt)
        ACT.activation(s_sig[:], psrc[:, 0:384], AF.Sigmoid).then_inc(s_sAX, 1)
        ACT.activation(s_tg[:], psrc[:, 384:512], AF.Tanh).then_inc(s_sBX, 1)
        DVE.wait_ge(s_sAX, tt)
        DVE.scalar_tensor_tensor(s_tA[:], s_sig[:, 128:256], 1.0, s_c[:],
                                 ALU.mult, ALU.mult)
        DVE.wait_ge(s_sBX, tt)
        DVE.scalar_tensor_tensor(s_tB[:], s_sig[:, 0:128], 1.0, s_tg[:],
                                 ALU.mult, ALU.mult)
        DVE.scalar_tensor_tensor(s_c[:], s_tA[:], 1.0, s_tB[:],
                                 ALU.mult, ALU.add).then_inc(s_cX, 1)
        ACT.wait_ge(s_cX, tt)
        ACT.activation(s_th[:], s_c[:], AF.Tanh).then_inc(s_thX, 1)
        DVE.wait_ge(s_thX, tt)
        DVE.scalar_tensor_tensor(s_h[:], s_sig[:, 256:384], 1.0, s_th[:],
                                 ALU.mult, ALU.mult).then_inc(s_hX, 1)

    def outproj(h2Tsrc, ks, start, stop, bias=False):
        for k in ks:
            for (o, w) in OPN:
                PE.matmul(p_op[:, o:o + w], h2Tsrc[:, k, :],
                          s_wout[:, k, o:o + w],
                          start=(start and k == ks[0]), stop=False)
        if bias:
            for (o, w) in OPN:
                PE.matmul(p_op[:, o:o + w], s_idn[:], s_boutb[:, o:o + w],
                          start=False, stop=True)

    for t in range(T_loop):
        h2Tprev = s_h2T[(t - 1) % 2]
        # ---- PE: g1 ----
        if t == 0:
            PE.wait_ge(s_bh0Tev, KT)
            PE.wait_ge(s_bc0ev, 1)
            PE.wait_ge(s_preEvA, 1)
            PE.wait_ge(s_preEvD, 1)
            PE.wait_ge(s_ldB, counts["s_ldB"])
        if t >= 2:
            PE.wait_ge(s_plEv, t - 1)
        if t > 0:
            PE.wait_ge(s_hT2, 32 * (t - 1) + 16)
        for k in range(4):
            PE.matmul(p_g1[:], h2Tprev[:, k, :], s_whh1[:, k, :],
                      start=(k == 0), stop=False)
        if t > 0:
            PE.wait_ge(s_hT2, 32 * t)
        for k in range(4, KT):
            PE.matmul(p_g1[:], h2Tprev[:, k, :], s_whh1[:, k, :],
                      start=False, stop=False)
        PE.matmul(p_g1[:], s_idn[:], s_pre[:, t, :], start=False, stop=True)
        PE.drain().then_inc(s_g1, 1)

        eltwise(p_g1, s_g1, s_sA1, s_sB1, s_c1, s_th1, s_h1, t)

        # ---- PE: out-proj part 1 for step t-1 (fills the eltwise-1 gap) ----
        if t == 1:
            PE.wait_ge(s_ldC, counts["s_ldC"])
        if t > 0:
            PE.wait_ge(s_oevA, t - 1)
            PE.wait_ge(s_oevD, t - 1)
            outproj(h2Tprev, [0, 1], start=True, stop=False)

        # ---- PE: transpose h1, DVE evac, AG1 ----
        PE.wait_ge(s_h1, t + 1)
        PE.transpose(p_t1[:], s_h[:], s_idnf[:])
        PE.drain().then_inc(s_tp1, 1)
        DVE.wait_ge(s_tp1, t + 1)
        DVE.tensor_copy(s_hcT1[:], p_t1[:]).then_inc(s_ev1, 1)
        SP.wait_ge(s_ev1, t + 1)
        if t > 0:
            SP.wait_ge(s_ag1, t)
        SP.dma_start(bin1[:], s_hcT1[:]).then_inc(s_din1, 16)
        PL.wait_ge(s_din1, 16 * (t + 1))
        PL.collective_compute(
            "AllGather", ALU.bypass, replica_groups=[list(range(NC))],
            ins=[bin1.opt()], outs=[bout1.opt()]).then_inc(s_ag1, 1)
        SP.wait_ge(s_ag1, t + 1)
        b1r = bout1.rearrange("(k p) b -> p k b", p=128)
        SP.dma_start(s_h1T[:, 0:4, :], b1r[:, 0:4, :]).then_inc(s_hT1, 16)
        SP.dma_start(s_h1T[:, 4:8, :], b1r[:, 4:8, :]).then_inc(s_hT1, 16)

        # ---- PE: out-proj part 2 for step t-1 (fills the AG1 window) ----
        if t > 0:
            outproj(h2Tprev, [2, 3, 4, 5], start=False, stop=False)

        # ---- PE: g2 ----
        PE.wait_ge(s_hT1, 32 * t + 16)
        for k in range(4):
            PE.matmul(p_g2[:], s_h1T[:, k, :], s_w2[:, k, :],
                      start=(k == 0), stop=False)
        PE.wait_ge(s_hT1, 32 * (t + 1))
        for k in range(4, KT):
            PE.matmul(p_g2[:], s_h1T[:, k, :], s_w2[:, k, :],
                      start=False, stop=False)
        PE.matmul(p_g2[:], s_idn[:], s_b2b[:], start=False, stop=True)
        PE.drain().then_inc(s_g2, 1)

        eltwise(p_g2, s_g2, s_sA2, s_sB2, s_c2, s_th2, s_h2, t)

        # ---- PE: out-proj part 3 for step t-1 (fills the eltwise-2 gap) ----
        if t > 0:
            outproj(h2Tprev, [6, 7], start=False, stop=True, bias=True)
            PE.drain().then_inc(s_op, 1)
            ACT.wait_ge(s_op, t)
            if t > 1:
                ACT.wait_ge(s_odma, 16 * (t - 1))
            ACT.activation(s_out[:, 0:768], p_op[:, 0:768],
                           AF.Copy).then_inc(s_oevA, 1)
            DVE.wait_ge(s_op, t)
            if t > 1:
                DVE.wait_ge(s_odma, 16 * (t - 1))
            DVE.tensor_copy(s_out[:, 768:VC], p_op[:, 768:VC]).then_inc(
                s_oevD, 1)
            SP.wait_ge(s_oevA, t)
            SP.wait_ge(s_oevD, t)
            SP.dma_start(y[(t - 1) * 128:t * 128, :], s_out[:]).then_inc(
                s_odma, 16)

        # ---- PE: transpose h2, AG2 ----
        PE.wait_ge(s_h2, t + 1)
        PE.transpose(p_t2[:], s_h[:], s_idnf[:])
        PE.drain().then_inc(s_tp2, 1)
        DVE.wait_ge(s_tp2, t + 1)
        DVE.tensor_copy(s_hcT2[:], p_t2[:]).then_inc(s_ev2, 1)
        SP.wait_ge(s_ev2, t + 1)
        if t > 0:
            SP.wait_ge(s_ag2, t)
        SP.dma_start(bin2[:], s_hcT2[:]).then_inc(s_din2, 16)
        PL.wait_ge(s_din2, 16 * (t + 1))
        PL.collective_compute(
            "AllGather", ALU.bypass, replica_groups=[list(range(NC))],
            ins=[bin2.opt()], outs=[bout2.opt()]).then_inc(s_ag2, 1)
        SP.wait_ge(s_ag2, t + 1)
        b2r = bout2.rearrange("(k p) b -> p k b", p=128)
        SP.dma_start(s_h2T[t % 2][:, 0:4, :], b2r[:, 0:4, :]).then_inc(
            s_hT2, 16)
        SP.dma_start(s_h2T[t % 2][:, 4:8, :], b2r[:, 4:8, :]).then_inc(
            s_hT2, 16)

        # ---- PE: pre-tile t+2 in the AG2 window ----
        if t + 2 < NT:
            m = t + 2
            if t > 0:
                PE.wait_ge(s_plEv, t)
            for k in range(ET):
                PE.matmul(p_pl[:], s_cap[:, k, m * 128:(m + 1) * 128],
                          s_we[:, k, :], start=(k == 0), stop=False)
            PE.matmul(p_pl[:], s_idn[:], s_img[:], start=False, stop=True)
            PE.drain().then_inc(s_plMM, 1)
            ACT.wait_ge(s_plMM, t + 1)
            ACT.activation(s_pre[:, m, :], p_pl[:], AF.Copy).then_inc(
                s_plEv, 1)

    # epilogue: out-proj for t = T-1
    PE.wait_ge(s_oevA, T_loop - 1)
    PE.wait_ge(s_oevD, T_loop - 1)
    last = s_h2T[(T_loop - 1) % 2]
    PE.wait_ge(s_hT2, 32 * T_loop)
    outproj(last, list(range(KT)), start=True, stop=True, bias=True)
    PE.drain().then_inc(s_op, 1)
    ACT.wait_ge(s_op, T_loop)
    ACT.wait_ge(s_odma, 16 * (T_loop - 1))
    ACT.activation(s_out[:, 0:768], p_op[:, 0:768], AF.Copy).then_inc(s_oevA, 1)
    DVE.wait_ge(s_op, T_loop)
    DVE.wait_ge(s_odma, 16 * (T_loop - 1))
    DVE.tensor_copy(s_out[:, 768:VC], p_op[:, 768:VC]).then_inc(s_oevD, 1)
    SP.wait_ge(s_oevA, T_loop)
    SP.wait_ge(s_oevD, T_loop)
    SP.dma_start(y[(T_loop - 1) * 128:T_loop * 128, :], s_out[:]).then_inc(
        s_odma, 16)
    SP.wait_ge(s_odma, 16 * T_loop)

    ctx.close()
    return nc


def _prepare_in_maps(image_vectors, captions_ix, W_img, b_img, emb, Wa, ba,
                     Ua, ub, va, vb, W_ih1, W_hh1, b_ih1, b_hh1, W_ih2, W_hh2,
                     b_ih2, b_hh2, W_out, b_out):
    f32 = np.float32
    IV = np.asarray(image_vectors, f32)
    cap = np.asarray(captions_ix).astype(np.int64)
    W_ih1 = np.asarray(W_ih1, f32); W_hh1 = np.asarray(W_hh1, f32)
    W2 = np.asarray(W_ih2, f32) + np.asarray(W_hh2, f32)
    b1 = np.asarray(b_ih1, f32) + np.asarray(b_hh1, f32)
    b2 = np.asarray(b_ih2, f32) + np.asarray(b_hh2, f32)
    W_out = np.asarray(W_out, f32); b_out = np.asarray(b_out, f32)
    W_img = np.asarray(W_img, f32); b_img = np.asarray(b_img, f32)
    emb_ = np.asarray(emb, f32)

    def rows(c):
        idx = []
        for gsel in (0, 1, 3, 2):  # torch (i,f,g,o) -> ours (i,f,o,g)
            base = gsel * H + c * HC
            idx.extend(range(base, base + HC))
        return np.array(idx)

    ce = emb_[cap.T.reshape(-1)]                       # [(t,b), E]
    capT_np = _kchunks(ce.T.astype(BF), B * T)
    ivT_np = _kchunks(IV.T.astype(BF), 128)
    wimgT_np = _kchunks(W_img.T.astype(BF), H)
    bimgb_np = np.broadcast_to(b_img.astype(BF), (128, H)).copy()
    eye_bf = np.eye(128, dtype=f32).astype(BF)
    eye_f32 = np.eye(128, dtype=f32)

    in_maps = []
    for c in range(NC):
        r = rows(c)
        W1c = W_ih1[r]
        hr = slice(c * HC, (c + 1) * HC)
        in_maps.append({
            "whh1T": _kchunks(W_hh1[r].T.astype(BF), G),
            "w2T": _kchunks(W2[r].T.astype(BF), G),
            "woutT": _kchunks(W_out[c * VC:(c + 1) * VC].T.astype(BF), VC),
            "weT": _kchunks(W1c[:, :E].T.astype(BF), G),
            "wfT": _kchunks(W1c[:, E:].T.astype(BF), G),
            "wimgT": wimgT_np,
            "wimgcT": _kchunks(W_img[hr].T.astype(BF), 128),
            "ivT": ivT_np,
            "capT": capT_np,
            "b1b": np.broadcast_to(b1[r].astype(BF), (128, G)).copy(),
            "b2b": np.broadcast_to(b2[r].astype(BF), (128, G)).copy(),
            "bimgb": bimgb_np,
            "bimgcb": np.broadcast_to(b_img[hr].astype(BF), (128, 128)).copy(),
            "boutb": np.broadcast_to(
                b_out[c * VC:(c + 1) * VC].astype(BF), (128, VC)).copy(),
            "idn": eye_bf,
            "idnf": eye_f32,
        })
    return in_maps


def kernel(**inputs):
    in_maps = _prepare_in_maps(**inputs)
    nc = bass.Bass("TRN2", target_bir_lowering=False, debug=False,
                   num_devices=NC)
    _build(nc)
    res = run_bass_kernel_spmd(nc, in_maps, core_ids=list(range(NC)))

    out = np.empty((B, T, V), np.float32)
    for c in range(NC):
        yc = res.results[c]["y"].reshape(T, B, VC)
        out[:, :, c * VC:(c + 1) * VC] = yc.transpose(1, 0, 2)
    return out
